# revision 23
# baseline (speedup 1.0000x reference)
"""MoE expert-parallel MLP kernel for Trainium2 (8 NeuronCores).

Problem: x:(1,8,2048,2048) f32, wi:(8,2048,4096), wo:(8,4096,2048)
         out = gelu_exact(x @ wi) @ wo   (per expert)

Sharding: expert parallelism — core e handles expert e entirely. No
collectives. Per-core math (C=2048 tokens, H=2048 hidden, I=4096 inter):

  GEMM1 (Strassen-1): h1[I, C] = wi[H, I].T @ xT[H, C]
  gelu:  h1 = gelu(h1)                       (ScalarE, exact erf gelu)
  GEMM2: out[C, H] = h1[I, C].T @ wo[I, H]   (lhsT = h1, natural layout)

All matmul operands are bf16 (PE 1 cyc/row; end-to-end rel err ~5e-3 vs
the 2e-2 gate). GEMM1 uses one level of Strassen over 2x2 blocks of
(I, H) x (H, C): 7 half-size products = 7/8 the PE rows of the plain
GEMM. Both operand combination sets are formed on the HOST (wi and xT
are kernel inputs, so their Strassen combos cost no device time); the
device pays only the output recombination adds, which run on
ScalarE+VectorE+Pool in the shadow of the next position's matmuls
(ScalarE copies the two doubly-used products out of PSUM, VectorE does
every PSUM-reading add — at most one PSUM operand per instruction —
and Pool the SBUF-only ones). The gelu drain then writes h1 as bf16.

Phasing: the C/2-wide quadrant-column space is processed in two halves
S (tokens S*512..+512 and 1024+S*512..+512); each phase runs
GEMM1-Strassen then plain GEMM2 for those 1024 tokens, so h1 stays
SBUF-resident at 64 KiB/partition (no DRAM round-trip, no on-device
transposes — the host pre-transposes x into the combo matrices).

PSUM: pool slots are bank-granular, so each Strassen position packs its
7 [128,256] products into the halves of 4 full banks, ping-ponging two
positions across the 8 banks. GEMM2 uses 4-bank co-quad groups at
N=512 with the same ping-pong.
"""
import numpy as np
from contextlib import ExitStack

import ml_dtypes
import concourse.bass as bass
import concourse.tile as tile
from concourse import bacc, mybir
from concourse.bass_utils import run_bass_kernel_spmd

P = 128
C, H, I = 2048, 2048, 4096
E = 8
F32 = mybir.dt.float32
BF16 = mybir.dt.bfloat16

H2, I2, C2 = H // 2, I // 2, C // 2   # 1024, 2048, 1024
K8 = H2 // P       # 8 k-subtiles per Strassen product
IB = I // P        # 32 GEMM2 k-subtiles
NQ = 256           # Strassen product free width (half bank)
N5 = 512
AL = mybir.AluOpType


def _build():
    nc = bacc.Bacc("TRN2", target_bir_lowering=False, debug=False, num_devices=E)
    # wa: host-pretiled lhsT combos; row (p*16+io)*128+pp, col k*128+i2
    wa = nc.dram_tensor("wa", [7 * 16 * P, K8 * P], BF16, kind="ExternalInput").ap()
    xb = nc.dram_tensor("xb", [7 * H2, C2], BF16, kind="ExternalInput").ap()
    wo = nc.dram_tensor("wo", [I, H], BF16, kind="ExternalInput").ap()
    out = nc.dram_tensor("out", [C, H], F32, kind="ExternalOutput").ap()

    GELU = mybir.ActivationFunctionType.Gelu

    with tile.TileContext(nc) as tc, ExitStack() as ctx:
        h1pool = ctx.enter_context(tc.tile_pool(name="h1", bufs=1))
        wapool = ctx.enter_context(tc.tile_pool(name="wa", bufs=14))
        xbpool = ctx.enter_context(tc.tile_pool(name="xb", bufs=8))
        wopool = ctx.enter_context(tc.tile_pool(name="wo", bufs=4))
        stage = ctx.enter_context(tc.tile_pool(name="stage", bufs=8))
        opool = ctx.enter_context(tc.tile_pool(name="outs", bufs=4))
        psum = ctx.enter_context(tc.tile_pool(name="psum", bufs=8, space="PSUM"))

        wa_t = {}

        def load_wa(S, io, p):
            # [128, 8k, 128i']: one io column-block of combo p (2KB runs)
            t = wapool.tile([P, K8, P], BF16, tag="wa", name=f"wa_{S}_{io}_{p}")
            nc.sync.dma_start(
                t[:],
                wa[(p * 16 + io) * P:(p * 16 + io + 1) * P, :]
                .rearrange("pp (k i) -> pp k i", k=K8))
            wa_t[(S, io, p)] = t

        xb_t = {}

        def load_xb(S, p, split=False):
            # [128, 8k, 512c'']: both cg halves of phase S (1KB runs)
            t = xbpool.tile([P, K8, N5], BF16, tag="xb", name=f"xb_{S}_{p}")
            src = xb[p * H2:(p + 1) * H2, S * N5:(S + 1) * N5] \
                .rearrange("(k pp) c -> pp k c", pp=P)
            if split:
                nc.sync.dma_start(t[:, :, 0:NQ], src[:, :, 0:NQ])
            else:
                nc.sync.dma_start(t[:], src)
            xb_t[(S, p)] = t
            return t, src

        wo_t = {}

        def load_wo(S, gq):
            ho, q = gq // 8, gq % 4
            t = wopool.tile([P, 8, N5], BF16, tag="wo", name=f"wo_{S}_{gq}")
            nc.sync.dma_start(
                t[:],
                wo[q * 8 * P:(q + 1) * 8 * P, ho * N5:(ho + 1) * N5]
                .rearrange("(s pp) h -> pp s h", pp=P))
            wo_t[(S, gq)] = t

        # ---- ramp: phase-0 xb set (cg0 halves first) + first wa block ----
        xb0_fin = []
        for p in range(7):
            t, src = load_xb(0, p, split=True)
            xb0_fin.append((t, src))
            load_wa(0, 0, p)
        for t, src in xb0_fin:
            nc.sync.dma_start(t[:, :, NQ:2 * NQ], src[:, :, NQ:2 * NQ])

        for S in range(2):
            # ---------- GEMM1 Strassen half-phase ----------
            h1 = h1pool.tile([P, IB, 1024], BF16, tag="h1", name=f"h1_{S}")
            for io in range(16):
                for cg in range(2):
                    # prefetch next io block (3-4 tiles per position);
                    # cross-phase prefetches happen in GEMM2 instead
                    # (pool FIFO order would otherwise deadlock)
                    if io + 1 < 16:
                        for pp in range(cg * 4, min(cg * 4 + 4, 7)):
                            if (S, io + 1, pp) not in wa_t:
                                load_wa(S, io + 1, pp)
                    if io == 15 and cg == 1:
                        load_wo(S, 0)
                        load_wo(S, 1)
                    # 7 products in the halves of 4 psum banks
                    mt = [psum.tile([P, N5], F32, tag="mm",
                                    name=f"m_{S}_{io}_{cg}_{j}")
                          for j in range(4)]
                    ms = [mt[p // 2][:, (p % 2) * NQ:(p % 2 + 1) * NQ]
                          for p in range(7)]
                    for p in range(7):
                        wt = wa_t[(S, io, p)]
                        xt = xb_t[(S, p)]
                        for k in range(K8):
                            nc.tensor.matmul(
                                ms[p], wt[:, k, :],
                                xt[:, k, cg * NQ:(cg + 1) * NQ],
                                start=(k == 0), stop=(k == K8 - 1))
                    # recombination on DVE+Pool. HW constraint: each op may
                    # read at most ONE PSUM operand, so chains go through
                    # SBUF intermediates (u=M1, a=M1+M4, x=M5, c=M1-M2) and
                    # t21 = a - c = M2+M4 reuses them for free.
                    def st(nm):
                        return stage.tile([P, NQ], F32, tag="st",
                                          name=f"{nm}_{S}_{io}_{cg}")
                    u = st("u"); a = st("a"); x = st("x"); b_ = st("b")
                    c_ = st("c"); d_ = st("d")
                    t11 = st("t11"); t12 = st("t12")
                    t21 = st("t21"); t22 = st("t22")
                    nc.scalar.copy(u[:], ms[0])                   # M1 (ACT)
                    nc.scalar.copy(x[:], ms[4])                   # M5 (ACT)
                    nc.vector.tensor_add(a[:], u[:], ms[3])       # M1+M4
                    nc.vector.tensor_add(b_[:], a[:], ms[6])      # M1+M4+M7
                    nc.vector.tensor_add(t12[:], x[:], ms[2])     # M5+M3
                    nc.vector.scalar_tensor_tensor(
                        c_[:], ms[1], -1.0, u[:], AL.mult, AL.add)  # M1-M2
                    nc.vector.tensor_add(d_[:], c_[:], ms[2])     # +M3
                    nc.vector.tensor_add(t22[:], d_[:], ms[5])    # +M6
                    nc.gpsimd.tensor_sub(t11[:], b_[:], x[:])     # SBUF only
                    nc.gpsimd.tensor_sub(t21[:], a[:], c_[:])     # M2+M4
                    # gelu drains into h1 (local cols [0:512]=C1 tokens,
                    # [512:1024]=C2 tokens)
                    lo = cg * NQ
                    nc.scalar.activation(h1[:, io, lo:lo + NQ], t11[:], GELU)
                    nc.scalar.activation(h1[:, io, 512 + lo:512 + lo + NQ],
                                         t12[:], GELU)
                    nc.scalar.activation(h1[:, 16 + io, lo:lo + NQ], t21[:], GELU)
                    nc.scalar.activation(h1[:, 16 + io, 512 + lo:512 + lo + NQ],
                                         t22[:], GELU)

            # ---------- GEMM2 for this phase's 1024 tokens ----------
            for ho in range(4):
                for qg in range(2):
                    if S == 0 and ho == 0 and qg == 0:
                        for p in range(7):
                            load_xb(1, p)
                    if S == 0 and ho == 2 and qg == 0:
                        for p in range(7):
                            load_wa(1, 0, p)
                    pss = [psum.tile([P, N5], F32, tag="mm",
                                     name=f"ps2_{S}_{ho}_{qg}_{c4}")
                           for c4 in range(4)]
                    base = S * N5 if qg == 0 else 1024 + S * N5

                    def drain(c4):
                        ot = opool.tile([P, N5], F32, tag="outs",
                                        name=f"o_{S}_{ho}_{qg}_{c4}")
                        nc.vector.tensor_copy(ot[:], pss[c4][:])
                        nc.scalar.dma_start(
                            out[base + c4 * P:base + (c4 + 1) * P,
                                ho * N5:(ho + 1) * N5], ot[:])

                    if S == 1 and ho == 3 and qg == 1:
                        # final group: two c4-pair passes over ik so the
                        # first pair's stores hide under the second pass
                        for half in range(2):
                            for q in range(4):
                                gq = 28 + q
                                key = (S, gq) if half == 0 and q < 2 else                                     (S, gq, half)
                                if key not in wo_t:
                                    ho_, q_ = gq // 8, gq % 4
                                    t = wopool.tile([P, 8, N5], BF16, tag="wo",
                                                    name=f"wo_l_{half}_{q}")
                                    nc.sync.dma_start(
                                        t[:],
                                        wo[q_ * 8 * P:(q_ + 1) * 8 * P,
                                           ho_ * N5:(ho_ + 1) * N5]
                                        .rearrange("(s pp) h -> pp s h", pp=P))
                                    wo_t[key] = t
                                wt = wo_t.pop(key)
                                for s8 in range(8):
                                    ik = q * 8 + s8
                                    for c4 in (half * 2, half * 2 + 1):
                                        nc.tensor.matmul(
                                            pss[c4][:],
                                            h1[:, ik, qg * N5 + c4 * P:
                                               qg * N5 + (c4 + 1) * P],
                                            wt[:, s8, :],
                                            start=(ik == 0), stop=(ik == IB - 1))
                            for c4 in (half * 2, half * 2 + 1):
                                drain(c4)
                    else:
                        for q in range(4):
                            gq = (ho * 2 + qg) * 4 + q
                            if gq + 2 < 32 and not (S == 1 and gq + 2 >= 30):
                                load_wo(S, gq + 2)
                            wt = wo_t.pop((S, gq))
                            for s8 in range(8):
                                ik = q * 8 + s8
                                for c4 in range(4):
                                    nc.tensor.matmul(
                                        pss[c4][:],
                                        h1[:, ik,
                                           qg * N5 + c4 * P:qg * N5 + (c4 + 1) * P],
                                        wt[:, s8, :],
                                        start=(ik == 0), stop=(ik == IB - 1))
                        for c4 in range(4):
                            drain(c4)

    nc.compile()
    return nc


_NC = None


def _host_prep(x, wi, wo):
    """Per-expert Strassen operand combos + bf16 casts (host side)."""
    bf = ml_dtypes.bfloat16
    xT = np.ascontiguousarray(np.swapaxes(x, 1, 2))      # [E, H, C]
    w11 = wi[:, :H2, :I2]; w12 = wi[:, :H2, I2:]
    w21 = wi[:, H2:, :I2]; w22 = wi[:, H2:, I2:]
    # lhsT combos, product order M1..M7
    was = [w11 + w22, w12 + w22, w11, w22, w11 + w21, w12 - w11, w21 - w22]
    # pre-tile each combo [1024, 2048] -> [16io*128pp, 8k*128i2] (2KB runs)
    wa = np.stack(
        [np.ascontiguousarray(
            c.reshape(E, K8, P, 16, P)
            .transpose(0, 3, 2, 1, 4).reshape(E, 16 * P, K8 * P))
         for c in was], axis=1).reshape(E, 7 * 16 * P, K8 * P).astype(bf)
    b11 = xT[:, :H2, :C2]; b12 = xT[:, :H2, C2:]
    b21 = xT[:, H2:, :C2]; b22 = xT[:, H2:, C2:]
    xbs = [b11 + b22, b11, b12 - b22, b21 - b11, b22, b11 + b12, b21 + b22]
    xbc = np.concatenate(xbs, axis=1).astype(bf)         # [E, 7*H2, C2]
    return wa, xbc, wo.astype(bf)


def kernel(x, wi, wo):
    global _NC
    if _NC is None:
        _NC = _build()
    x = np.asarray(x, dtype=np.float32).reshape(E, C, H)
    wi = np.ascontiguousarray(np.asarray(wi, dtype=np.float32))
    wo = np.ascontiguousarray(np.asarray(wo, dtype=np.float32))
    wa, xbc, wob = _host_prep(x, wi, wo)
    in_maps = [{"wa": wa[e], "xb": xbc[e], "wo": wob[e]} for e in range(E)]
    res = run_bass_kernel_spmd(_NC, in_maps, core_ids=list(range(E)))
    o = np.stack([res.results[e]["out"] for e in range(E)])[None]
    return o


# revision 29
# speedup vs baseline: 1.0057x; 1.0057x over previous
"""MoE expert-parallel MLP kernel for Trainium2 (8 NeuronCores).

Problem: x:(1,8,2048,2048) f32, wi:(8,2048,4096), wo:(8,4096,2048)
         out = gelu_exact(x @ wi) @ wo   (per expert)

Sharding: expert parallelism — core e handles expert e entirely. No
collectives. Per-core math (C=2048 tokens, H=2048 hidden, I=4096 inter):

  GEMM1 (Strassen-1): h1[I, C] = wi[H, I].T @ xT[H, C]
  gelu:  h1 = gelu(h1)                       (ScalarE, exact erf gelu)
  GEMM2: out[C, H] = h1[I, C].T @ wo[I, H]   (lhsT = h1, natural layout)

All matmul operands are bf16 (PE 1 cyc/row; end-to-end rel err ~5e-3 vs
the 2e-2 gate). GEMM1 uses one level of Strassen over 2x2 blocks of
(I, H) x (H, C): 7 half-size products = 7/8 the PE rows of the plain
GEMM. Both operand combination sets are formed on the HOST (wi and xT
are kernel inputs, so their Strassen combos cost no device time); the
device pays only the output recombination adds, which run on
ScalarE+VectorE+Pool in the shadow of the next position's matmuls
(ScalarE copies the two doubly-used products out of PSUM, VectorE does
every PSUM-reading add — at most one PSUM operand per instruction —
and Pool the SBUF-only ones). The gelu drain then writes h1 as bf16.

Phasing: the C/2-wide quadrant-column space is processed in two halves
S (tokens S*512..+512 and 1024+S*512..+512); each phase runs
GEMM1-Strassen then plain GEMM2 for those 1024 tokens, so h1 stays
SBUF-resident at 64 KiB/partition (no DRAM round-trip, no on-device
transposes — the host pre-transposes x into the combo matrices).

PSUM: pool slots are bank-granular, so each Strassen position packs its
7 [128,256] products into the halves of 4 full banks, ping-ponging two
positions across the 8 banks. GEMM2 uses 4-bank co-quad groups at
N=512 with the same ping-pong.
"""
import numpy as np
from contextlib import ExitStack

import ml_dtypes
import concourse.bass as bass
import concourse.tile as tile
from concourse import bacc, mybir
from concourse.bass_utils import run_bass_kernel_spmd

P = 128
C, H, I = 2048, 2048, 4096
E = 8
F32 = mybir.dt.float32
BF16 = mybir.dt.bfloat16

H2, I2, C2 = H // 2, I // 2, C // 2   # 1024, 2048, 1024
K8 = H2 // P       # 8 k-subtiles per Strassen product
IB = I // P        # 32 GEMM2 k-subtiles
NQ = 256           # Strassen product free width (half bank)
N5 = 512
AL = mybir.AluOpType


def _build():
    nc = bacc.Bacc("TRN2", target_bir_lowering=False, debug=False, num_devices=E)
    # wa: host-pretiled lhsT combos; row (p*16+io)*128+pp, col k*128+i2
    wa = nc.dram_tensor("wa", [7 * 16 * P, K8 * P], BF16, kind="ExternalInput").ap()
    xb = nc.dram_tensor("xb", [7 * H2, C2], BF16, kind="ExternalInput").ap()
    wo = nc.dram_tensor("wo", [I, H], BF16, kind="ExternalInput").ap()
    out = nc.dram_tensor("out", [C, H], F32, kind="ExternalOutput").ap()

    GELU = mybir.ActivationFunctionType.Gelu

    with tile.TileContext(nc) as tc, ExitStack() as ctx:
        h1pool = ctx.enter_context(tc.tile_pool(name="h1", bufs=1))
        wapool = ctx.enter_context(tc.tile_pool(name="wa", bufs=14))
        xbpool = ctx.enter_context(tc.tile_pool(name="xb", bufs=8))
        wopool = ctx.enter_context(tc.tile_pool(name="wo", bufs=4))
        stage = ctx.enter_context(tc.tile_pool(name="stage", bufs=8))
        opool = ctx.enter_context(tc.tile_pool(name="outs", bufs=4))
        psum = ctx.enter_context(tc.tile_pool(name="psum", bufs=8, space="PSUM"))

        wa_t = {}

        def load_wa(S, io, p):
            # [128, 8k, 128i']: one io column-block of combo p (2KB runs)
            t = wapool.tile([P, K8, P], BF16, tag="wa", name=f"wa_{S}_{io}_{p}")
            nc.sync.dma_start(
                t[:],
                wa[(p * 16 + io) * P:(p * 16 + io + 1) * P, :]
                .rearrange("pp (k i) -> pp k i", k=K8))
            wa_t[(S, io, p)] = t

        xb_t = {}

        def load_xb(S, p, split=False):
            # [128, 8k, 512c'']: both cg halves of phase S (1KB runs)
            t = xbpool.tile([P, K8, N5], BF16, tag="xb", name=f"xb_{S}_{p}")
            src = xb[p * H2:(p + 1) * H2, S * N5:(S + 1) * N5] \
                .rearrange("(k pp) c -> pp k c", pp=P)
            if split:
                nc.sync.dma_start(t[:, :, 0:NQ], src[:, :, 0:NQ])
            else:
                nc.sync.dma_start(t[:], src)
            xb_t[(S, p)] = t
            return t, src

        wo_t = {}

        def load_wo(S, gq):
            ho, q = gq // 8, gq % 4
            t = wopool.tile([P, 8, N5], BF16, tag="wo", name=f"wo_{S}_{gq}")
            nc.sync.dma_start(
                t[:],
                wo[q * 8 * P:(q + 1) * 8 * P, ho * N5:(ho + 1) * N5]
                .rearrange("(s pp) h -> pp s h", pp=P))
            wo_t[(S, gq)] = t

        # ---- ramp: phase-0 xb set (cg0 halves first) + first two wa
        # blocks, then the cg1 xb halves. The first four positions run in a
        # special order — (0,0), (1,0), then (0,1)+(1,1) product-interleaved
        # — so PE consumption keeps pace with each DMA stream.
        xb0_fin = []
        for p in range(7):
            if p == 0:
                # k-split the very first tile: the first matmuls only need
                # k 0-3, so PE starts ~1.5us sooner
                t = xbpool.tile([P, K8, N5], BF16, tag="xb", name="xb_0_0")
                src = xb[0:H2, 0:N5].rearrange("(k pp) c -> pp k c", pp=P)
                nc.sync.dma_start(t[:, 0:4, 0:NQ], src[:, 0:4, 0:NQ])
                load_wa(0, 0, 0)
                nc.sync.dma_start(t[:, 4:K8, 0:NQ], src[:, 4:K8, 0:NQ])
                xb_t[(0, 0)] = t
                xb0_fin.append((t, src))
                continue
            t, src = load_xb(0, p, split=True)
            xb0_fin.append((t, src))
            load_wa(0, 0, p)
        for p in range(7):
            load_wa(0, 1, p)
        for t, src in xb0_fin:
            nc.sync.dma_start(t[:, :, NQ:2 * NQ], src[:, :, NQ:2 * NQ])

        def alloc_ms(S, io, cg):
            mt = [psum.tile([P, N5], F32, tag="mm", name=f"m_{S}_{io}_{cg}_{j}")
                  for j in range(4)]
            return [mt[p // 2][:, (p % 2) * NQ:(p % 2 + 1) * NQ]
                    for p in range(7)]

        def g1_products(S, io, cg, ms):
            for p in range(7):
                wt = wa_t[(S, io, p)]
                xt = xb_t[(S, p)]
                for k in range(K8):
                    nc.tensor.matmul(ms[p], wt[:, k, :],
                                     xt[:, k, cg * NQ:(cg + 1) * NQ],
                                     start=(k == 0), stop=(k == K8 - 1))

        def g1_recombine(S, io, cg, ms, h1):
            # each op reads at most ONE PSUM operand (HW rule); ACT pulls
            # the two doubly-used products, DVE orders its chain so PSUM
            # banks free in allocation order (j0 first)
            def st(nm):
                return stage.tile([P, NQ], F32, tag="st",
                                  name=f"{nm}_{S}_{io}_{cg}")
            u = st("u"); a = st("a"); x = st("x"); b_ = st("b")
            c_ = st("c"); d_ = st("d")
            t11 = st("t11"); t12 = st("t12")
            t21 = st("t21"); t22 = st("t22")
            nc.scalar.copy(u[:], ms[0])                   # M1 (ACT)
            nc.scalar.copy(x[:], ms[4])                   # M5 (ACT)
            nc.vector.tensor_add(a[:], u[:], ms[3])       # M1+M4
            nc.vector.scalar_tensor_tensor(
                c_[:], ms[1], -1.0, u[:], AL.mult, AL.add)  # M1-M2
            nc.vector.tensor_add(b_[:], a[:], ms[6])      # M1+M4+M7
            nc.vector.tensor_add(t12[:], x[:], ms[2])     # M5+M3
            nc.vector.tensor_add(d_[:], c_[:], ms[2])     # +M3
            nc.vector.tensor_add(t22[:], d_[:], ms[5])    # +M6
            nc.gpsimd.tensor_sub(t21[:], a[:], c_[:])     # M2+M4 (SBUF only)
            nc.gpsimd.tensor_sub(t11[:], b_[:], x[:])     # SBUF only
            lo = cg * NQ
            nc.scalar.activation(h1[:, io, lo:lo + NQ], t11[:], GELU)
            nc.scalar.activation(h1[:, io, 512 + lo:512 + lo + NQ],
                                 t12[:], GELU)
            nc.scalar.activation(h1[:, 16 + io, lo:lo + NQ], t21[:], GELU)
            nc.scalar.activation(h1[:, 16 + io, 512 + lo:512 + lo + NQ],
                                 t22[:], GELU)

        for S in range(2):
            # ---------- GEMM1 Strassen half-phase ----------
            h1 = h1pool.tile([P, IB, 1024], BF16, tag="h1", name=f"h1_{S}")
            io_start = 0
            if S == 0:
                # ramp schedule: cg0 of io 0-1 sequentially (paced by the
                # xb-h1/wa streams), then cg1 of both interleaved per
                # product so two positions consume each xb-h2 arrival
                ms00 = alloc_ms(0, 0, 0)
                g1_products(0, 0, 0, ms00)
                g1_recombine(0, 0, 0, ms00, h1)
                ms10 = alloc_ms(0, 1, 0)
                g1_products(0, 1, 0, ms10)
                g1_recombine(0, 1, 0, ms10, h1)
                msA = alloc_ms(0, 0, 1)
                msB = alloc_ms(0, 1, 1)
                for p in range(7):
                    for io_, ms_ in ((0, msA), (1, msB)):
                        wt = wa_t[(0, io_, p)]
                        xt = xb_t[(0, p)]
                        for k in range(K8):
                            nc.tensor.matmul(ms_[p], wt[:, k, :],
                                             xt[:, k, NQ:2 * NQ],
                                             start=(k == 0), stop=(k == K8 - 1))
                    if p == 1:
                        for pp in range(4):
                            load_wa(0, 2, pp)
                    if p == 4:
                        for pp in range(4, 7):
                            load_wa(0, 2, pp)
                g1_recombine(0, 0, 1, msA, h1)
                g1_recombine(0, 1, 1, msB, h1)
                io_start = 2
            for io in range(io_start, 16):
                for cg in range(2):
                    # prefetch next io block (3-4 tiles per position);
                    # cross-phase prefetches happen in GEMM2 instead
                    # (pool FIFO order would otherwise deadlock)
                    if io + 1 < 16:
                        for pp in range(cg * 4, min(cg * 4 + 4, 7)):
                            if (S, io + 1, pp) not in wa_t:
                                load_wa(S, io + 1, pp)
                    if io == 15 and cg == 1:
                        load_wo(S, 0)
                        load_wo(S, 1)
                    # 7 products in the halves of 4 psum banks, then
                    # recombination + gelu drain (see helpers above)
                    ms = alloc_ms(S, io, cg)
                    g1_products(S, io, cg, ms)
                    g1_recombine(S, io, cg, ms, h1)

            # ---------- GEMM2 for this phase's 1024 tokens ----------
            for ho in range(4):
                for qg in range(2):
                    if S == 0 and ho == 0 and qg == 0:
                        for p in range(7):
                            load_xb(1, p)
                    if S == 0 and ho == 2 and qg == 0:
                        for p in range(7):
                            load_wa(1, 0, p)
                    pss = [psum.tile([P, N5], F32, tag="mm",
                                     name=f"ps2_{S}_{ho}_{qg}_{c4}")
                           for c4 in range(4)]
                    base = S * N5 if qg == 0 else 1024 + S * N5

                    def drain(c4):
                        ot = opool.tile([P, N5], F32, tag="outs",
                                        name=f"o_{S}_{ho}_{qg}_{c4}")
                        nc.vector.tensor_copy(ot[:], pss[c4][:])
                        nc.scalar.dma_start(
                            out[base + c4 * P:base + (c4 + 1) * P,
                                ho * N5:(ho + 1) * N5], ot[:])

                    if S == 1 and ho == 3 and qg == 1:
                        # final group: all 4 wo chunks stay live so each c4
                        # runs its full ik-pass alone and drains while the
                        # next c4 computes — only the last drain is exposed
                        wts = [wo_t.pop((S, 28 + q)) for q in range(4)]
                        for c4 in range(4):
                            for q in range(4):
                                for s8 in range(8):
                                    ik = q * 8 + s8
                                    nc.tensor.matmul(
                                        pss[c4][:],
                                        h1[:, ik, qg * N5 + c4 * P:
                                           qg * N5 + (c4 + 1) * P],
                                        wts[q][:, s8, :],
                                        start=(ik == 0), stop=(ik == IB - 1))
                            if c4 < 3:
                                drain(c4)
                            else:
                                # split the very last drain across engines
                                # and queues so its fixed latencies overlap
                                ot = opool.tile([P, N5], F32, tag="outs",
                                                name="o_last")
                                nc.vector.tensor_copy(ot[:, 0:NQ],
                                                      pss[3][:, 0:NQ])
                                nc.scalar.copy(ot[:, NQ:N5], pss[3][:, NQ:N5])
                                nc.scalar.dma_start(
                                    out[base + 3 * P:base + 4 * P,
                                        ho * N5:ho * N5 + NQ], ot[:, 0:NQ])
                                nc.sync.dma_start(
                                    out[base + 3 * P:base + 4 * P,
                                        ho * N5 + NQ:(ho + 1) * N5],
                                    ot[:, NQ:N5])
                    else:
                        for q in range(4):
                            gq = (ho * 2 + qg) * 4 + q
                            if gq + 2 < 32:
                                load_wo(S, gq + 2)
                            if S == 1 and ho == 3 and qg == 0 and q >= 2:
                                load_wo(S, 28 + q)  # final group's q=2,3
                            wt = wo_t.pop((S, gq))
                            for s8 in range(8):
                                ik = q * 8 + s8
                                for c4 in range(4):
                                    nc.tensor.matmul(
                                        pss[c4][:],
                                        h1[:, ik,
                                           qg * N5 + c4 * P:qg * N5 + (c4 + 1) * P],
                                        wt[:, s8, :],
                                        start=(ik == 0), stop=(ik == IB - 1))
                        for c4 in range(4):
                            drain(c4)

    nc.compile()
    return nc


_NC = None


def _host_prep(x, wi, wo):
    """Per-expert Strassen operand combos + bf16 casts (host side)."""
    bf = ml_dtypes.bfloat16
    xT = np.ascontiguousarray(np.swapaxes(x, 1, 2))      # [E, H, C]
    w11 = wi[:, :H2, :I2]; w12 = wi[:, :H2, I2:]
    w21 = wi[:, H2:, :I2]; w22 = wi[:, H2:, I2:]
    # lhsT combos, product order M1..M7
    was = [w11 + w22, w12 + w22, w11, w22, w11 + w21, w12 - w11, w21 - w22]
    # pre-tile each combo [1024, 2048] -> [16io*128pp, 8k*128i2] (2KB runs)
    wa = np.stack(
        [np.ascontiguousarray(
            c.reshape(E, K8, P, 16, P)
            .transpose(0, 3, 2, 1, 4).reshape(E, 16 * P, K8 * P))
         for c in was], axis=1).reshape(E, 7 * 16 * P, K8 * P).astype(bf)
    b11 = xT[:, :H2, :C2]; b12 = xT[:, :H2, C2:]
    b21 = xT[:, H2:, :C2]; b22 = xT[:, H2:, C2:]
    xbs = [b11 + b22, b11, b12 - b22, b21 - b11, b22, b11 + b12, b21 + b22]
    xbc = np.concatenate(xbs, axis=1).astype(bf)         # [E, 7*H2, C2]
    return wa, xbc, wo.astype(bf)


def kernel(x, wi, wo):
    global _NC
    if _NC is None:
        _NC = _build()
    x = np.asarray(x, dtype=np.float32).reshape(E, C, H)
    wi = np.ascontiguousarray(np.asarray(wi, dtype=np.float32))
    wo = np.ascontiguousarray(np.asarray(wo, dtype=np.float32))
    wa, xbc, wob = _host_prep(x, wi, wo)
    in_maps = [{"wa": wa[e], "xb": xbc[e], "wo": wob[e]} for e in range(E)]
    res = run_bass_kernel_spmd(_NC, in_maps, core_ids=list(range(E)))
    o = np.stack([res.results[e]["out"] for e in range(E)])[None]
    return o
